# revision 1
# baseline (speedup 1.0000x reference)
"""APPNP-over-GAT distributed Trainium2 kernel (8 NeuronCores), v2.

Sharding: tensor-parallel over (head, out_feat). Each core owns a 128-wide
slice of every head's 1024 out-features (3*128 = 384 local features).

v2 restructure vs v1 (246 us):
- L1-norm |x| column sums via vector tensor_reduce(apply_absolute_value)
  on [128, 4*512] group tiles: kills the 47 us scalar ABS pass.
- Softmax denominators fused into the h0 matmul by appending a ones
  column per head to the projected-h rhs tiles ([128, 3, 129] layout):
  no [1,512] den matmuls, no DRAM round-trip transpose.
- APPNP via matrix squaring: B=0.9*Ahat (transposed tiles), precompute
  (B^2)^T and (B^4)^T on the tensor engine DURING the el/er AllReduce
  wait; the k=10 chain becomes 5 matmul rounds instead of 10.
- Attention exp pipeline split across gpsimd (z=el+er), vector (lrelu),
  scalar (exp), gpsimd/vector (mask mul).
- fc partial dots via tensor_tensor_reduce (fused mul+reduce), final
  cross-partition sum on gpsimd (axis C) instead of a PSUM matmul.
- dinv / Ahat prep / fcw load all pulled off the critical path.
"""

import os
import sys

sys.path.insert(0, "/opt/trn_rl_repo")

import numpy as np

N = 500
NP = 512  # padded nodes
F = 8192
H = 3
O = 1024
OL = 128  # out-features per head per core
SH = H * OL  # 384 local features
KF = F // 128  # 64 k-tiles
G = 4  # k-tiles per DMA group
NG = KF // G  # 16 groups
XB = G * NP  # 2048: x block cols per group tile
WB = G * SH  # 1536: w block cols per group tile
NC = 8
K_STEPS = 10
ALPHA = 0.1
NEG_SLOPE = 0.2

LAST_EXEC_NS = None
LAST_RESULT = None


def build(stage=99):
    import concourse.bacc as bacc
    import concourse.mybir as mybir
    import concourse.tile as tile
    from concourse.masks import make_identity

    f32 = mybir.dt.float32
    bf16 = mybir.dt.bfloat16
    Alu = mybir.AluOpType
    Act = mybir.ActivationFunctionType
    AX = mybir.AxisListType.X
    AC = mybir.AxisListType.C

    nc = bacc.Bacc("TRN2", target_bir_lowering=False, debug=False, num_devices=NC)

    xw = nc.declare_dram_parameter("xw", [NG, 128, XB + WB], bf16, isOutput=False)
    aftp = nc.declare_dram_parameter("aftp", [128, 4 * NP], bf16, isOutput=False)
    afp = nc.declare_dram_parameter("afp", [128, 4 * NP], bf16, isOutput=False)
    attn = nc.declare_dram_parameter("attn", [128, 2 * SH], bf16, isOutput=False)
    fcwp = nc.declare_dram_parameter("fcwp", [128, 4 * 2 * SH], bf16, isOutput=False)
    fcb = nc.declare_dram_parameter("fcb", [1, 16], f32, isOutput=False)
    out_ext = nc.declare_dram_parameter("out", [1, 16], f32, isOutput=True)

    rg = [list(range(NC))]

    for _single_pass in range(1):
        with tile.TileContext(nc) as tc:
            with (
                tc.tile_pool(name="consts", bufs=1) as consts,
                tc.tile_pool(name="persist", bufs=1) as persist,
                tc.tile_pool(name="stream", bufs=3) as stream,
                tc.tile_pool(name="dram", bufs=1, space="DRAM") as dram,
            ):
                ident = consts.tile([128, 128], bf16, name="ident", tag="ident")
                make_identity(nc, ident[:, :])
                ones_col_b = consts.tile([128, 1], bf16, name="ones_col_b", tag="ocb")
                nc.gpsimd.memset(ones_col_b[:, :], 1.0)
                ones_col = consts.tile([128, 1], f32, name="ones_col", tag="ocf")
                nc.gpsimd.memset(ones_col[:, :], 1.0)
                ones_row = consts.tile([1, 128], f32, name="ones_row", tag="ones_row")
                nc.gpsimd.memset(ones_row[:, :], 1.0)

                # ---- warm-up collective: absorbs the CC engine's ~11us
                # cold-start latency and any cross-core launch skew, so the
                # real AllReduce later hits a warm CC pipeline.
                warm_in = dram.tile([1, 64], f32, name="warm_in", tag="warm_in")
                warm_out = dram.tile([1, 64], f32, name="warm_out", tag="warm_out")
                nc.gpsimd.collective_compute(
                    "AllReduce", Alu.add, ins=[warm_in.opt()], outs=[warm_out.opt()],
                    replica_groups=rg,
                )

                # ---- prologue loads (aft/attn on gpsimd queue, rest on scalar/HWDGE)
                attn_sb = consts.tile([128, 2 * SH], bf16, name="attn_sb", tag="attn_sb")
                nc.gpsimd.dma_start(attn_sb[:, :], attn[:, :])
                aft_sb = persist.tile([128, 4 * NP], bf16, name="aft_sb", tag="aft_sb")
                nc.gpsimd.dma_start(aft_sb[:, :], aftp[:, :])
                aft_t = [aft_sb[:, k * NP : (k + 1) * NP] for k in range(4)]
                af_sb = persist.tile([128, 4 * NP], bf16, name="af_sb", tag="af_sb")
                af_t = [af_sb[:, k * NP : (k + 1) * NP] for k in range(4)]
                fcb_sb = consts.tile([1, 16], f32, name="fcb_sb", tag="fcb_sb")
                fcw_sb = persist.tile([128, 8 * SH], bf16, name="fcw_sb", tag="fcw_sb")
                fcw_t = [fcw_sb[:, m * 2 * SH : (m + 1) * 2 * SH] for m in range(4)]

                ppA = tc.tile_pool(name="psumA", bufs=1, space="PSUM")
                pp = ppA.__enter__()

                # ---- fused projection stream: h = (x / l1colsum(x)) @ W
                # group tile: [128, x-tiles in order j0,j2,j1,j3 | w j0..j3]
                # XOFF[j]: x-block slot of logical k-tile j; SCOL[j]: s column
                XOFF = [0, 2, 1, 3]
                hp_psum = [pp.tile([128, SH], f32, name=f"hp{m}", tag=f"hp{m}") for m in range(4)]
                for g in range(NG):
                    xwt = stream.tile([128, XB + WB], bf16, name="xwt", tag="xwt")
                    nc.sync.dma_start(xwt[:, :], xw[g, :, :])
                    # |x| column sums: tiles j0/j2 (slots 0,1) on vector in one
                    # reduce; j1/j3 (slots 2,3) on scalar with fused abs+accum
                    s_g = stream.tile([128, 4], f32, name="sg", tag="sg", bufs=4)
                    nc.vector.tensor_reduce(
                        s_g[:, 0:2],
                        xwt[:, 0 : 2 * NP].rearrange("p (j n) -> p j n", j=2),
                        axis=AX, op=Alu.add, apply_absolute_value=True,
                    )
                    for slot in (2, 3):
                        absj = stream.tile([128, NP], bf16, name="absj", tag="absj", bufs=4)
                        nc.scalar.activation(
                            absj[:, :], xwt[:, slot * NP : (slot + 1) * NP],
                            Act.Abs, accum_out=s_g[:, slot : slot + 1],
                        )
                    rs_g = stream.tile([128, 4], f32, name="rsg", tag="rsg", bufs=4)
                    nc.vector.reciprocal(rs_g[:, 0:2], s_g[:, 0:2])
                    nc.vector.reciprocal(rs_g[:, 2:4], s_g[:, 2:4])
                    for j in range(G):
                        k = g * G + j
                        sc = XOFF[j]
                        wk = xwt[:, XB + j * SH : XB + (j + 1) * SH]
                        wks = stream.tile([128, SH], bf16, name="wks", tag="wks", bufs=6)
                        if j == 1 or j == 3:
                            nc.scalar.mul(wks[:, :], wk, rs_g[:, sc : sc + 1])
                        else:
                            nc.vector.tensor_scalar_mul(wks[:, :], wk, rs_g[:, sc : sc + 1])
                        for m in range(4):
                            nc.tensor.matmul(
                                hp_psum[m][:, :],
                                xwt[:, XOFF[j] * NP + m * 128 : XOFF[j] * NP + (m + 1) * 128],
                                wks[:, :],
                                start=(k == 0),
                                stop=(k == KF - 1),
                            )
                # late loads (needed only after the projection finishes)
                nc.sync.dma_start(af_sb[:, :], afp[:, :])
                nc.sync.dma_start(fcb_sb[:, :], fcb[:, :])
                nc.sync.dma_start(fcw_sb[:, :], fcwp[:, :])
                # ---- el/er partial dots straight from PSUM -> AllReduce
                eler_in = dram.tile([NP, 6], f32, name="eler_in", tag="eler_in")
                eler_out = dram.tile([NP, 6], f32, name="eler_out", tag="eler_out")
                for m in range(4):
                    eler_m = stream.tile([128, 6], f32, name="eler_m", tag="eler_m")
                    for s in range(2):
                        prod = stream.tile([128, SH], bf16, name="elprod", tag="elprod")
                        nc.vector.tensor_mul(
                            prod[:, :], hp_psum[m][:, :],
                            attn_sb[:, s * SH : (s + 1) * SH],
                        )
                        nc.vector.tensor_reduce(
                            eler_m[:, 3 * s : 3 * s + 3],
                            prod.rearrange("p (h o) -> p h o", h=H),
                            axis=AX, op=Alu.add,
                        )
                    nc.sync.dma_start(eler_in[m * 128 : (m + 1) * 128, :], eler_m[:, :])
                nc.gpsimd.collective_compute(
                    "AllReduce", Alu.add, ins=[eler_in.opt()], outs=[eler_out.opt()],
                    replica_groups=rg,
                )

                # h tiles with a ones column appended per head: [128, 3, 129]
                # (copied out during the AllReduce window)
                hp_sb = [
                    persist.tile([128, H, 129], bf16, name=f"hpsb{m}", tag=f"hpsb{m}")
                    for m in range(4)
                ]
                for m in range(4):
                    nc.gpsimd.memset(hp_sb[m][:, :, :], 1.0)
                    for h in range(H):
                        nc.scalar.copy(
                            hp_sb[m][:, h, 0:128],
                            hp_psum[m][:, h * OL : (h + 1) * OL],
                        )

                if stage <= 1:
                    resm = stream.tile([1, 16], f32, name="resm", tag="resm")
                    nc.gpsimd.memset(resm[:, :], 0.0)
                    nc.vector.tensor_copy(resm[:, 0:16], hp_sb[0][0:1, 0, 0:16])
                    nc.sync.dma_start(out_ext[:, :], resm[:, :])
                    ppA.__exit__(None, None, None)
                    break

                if stage <= 2:
                    erl2 = persist.tile([128, 4, 6], f32, name="erl2", tag="erl2")
                    nc.sync.dma_start(erl2[:, :, :], eler_out.rearrange("(k p) c -> p k c", p=128))
                    resm = stream.tile([1, 16], f32, name="resm", tag="resm")
                    nc.gpsimd.memset(resm[:, :], 0.0)
                    nc.vector.tensor_copy(resm[:, 0:6], erl2[0:1, 0, 0:6])
                    nc.sync.dma_start(out_ext[:, :], resm[:, :])
                    ppA.__exit__(None, None, None)
                    break

                # ---- degree/dinv (on-chip, no DRAM round trip)
                d_row = pp.tile([1, NP], f32, name="d_row", tag="d_row")
                for k in range(4):
                    nc.tensor.matmul(
                        d_row[:, :], ones_col_b[:, :], aft_t[k],
                        start=(k == 0), stop=(k == 3),
                    )
                dc_psum = pp.tile([128, 4], f32, name="dc_psum", tag="dc_psum")
                for m in range(4):
                    for k in range(4):
                        nc.tensor.matmul(
                            dc_psum[:, m : m + 1],
                            aft_t[k][:, m * 128 : (m + 1) * 128],
                            ones_col_b[:, :],
                            start=(k == 0), stop=(k == 3),
                        )
                sdr = stream.tile([1, NP], f32, name="sdr", tag="sdr")
                nc.scalar.activation(sdr[:, :], d_row[:, :], Act.Sqrt, bias=1.0)
                dinv_row = consts.tile([1, NP], f32, name="dinv_row", tag="dinv_row")
                nc.vector.reciprocal(dinv_row[:, :], sdr[:, :])
                sdc = stream.tile([128, 4], f32, name="sdc", tag="sdc")
                nc.scalar.activation(sdc[:, :], dc_psum[:, :], Act.Sqrt, bias=1.0)
                dinvc = persist.tile([128, 4], f32, name="dinvc", tag="dinvc")
                nc.vector.reciprocal(dinvc[:, :], sdc[:, :])
                dvb_psum = pp.tile([128, NP], f32, name="dvb_psum", tag="dvb_psum")
                nc.tensor.matmul(dvb_psum[:, :], ones_row[:, :], dinv_row[:, :])
                dinvb09 = persist.tile([128, NP], f32, name="dinvb09", tag="dinvb09")
                nc.scalar.mul(dinvb09[:, :], dvb_psum[:, :], 1.0 - ALPHA)

                # ---- B = 0.9*Ahat tiles (transposed + untransposed twins, bf16)
                bt_sb = persist.tile([128, 4 * NP], bf16, name="bt_sb", tag="bt_sb")
                bu_sb = persist.tile([128, 4 * NP], bf16, name="bu_sb", tag="bu_sb")
                bt_t = [bt_sb[:, k * NP : (k + 1) * NP] for k in range(4)]
                bu_t = [bu_sb[:, k * NP : (k + 1) * NP] for k in range(4)]
                for k in range(4):
                    # add I on the diagonal blocks (A_sl = A + I)
                    nc.vector.tensor_add(
                        aft_sb[:, k * NP + k * 128 : k * NP + (k + 1) * 128],
                        aft_sb[:, k * NP + k * 128 : k * NP + (k + 1) * 128],
                        ident[:, :],
                    )
                    nc.vector.tensor_add(
                        af_sb[:, k * NP + k * 128 : k * NP + (k + 1) * 128],
                        af_sb[:, k * NP + k * 128 : k * NP + (k + 1) * 128],
                        ident[:, :],
                    )
                    nc.vector.scalar_tensor_tensor(
                        bt_t[k], aft_t[k], dinvc[:, k : k + 1], dinvb09[:, :],
                        op0=Alu.mult, op1=Alu.mult,
                    )
                    nc.vector.scalar_tensor_tensor(
                        bu_t[k], af_t[k], dinvc[:, k : k + 1], dinvb09[:, :],
                        op0=Alu.mult, op1=Alu.mult,
                    )
                    # restore raw A^T for the attention mask (A_sl only for degrees)
                    nc.vector.tensor_sub(
                        aft_sb[:, k * NP + k * 128 : k * NP + (k + 1) * 128],
                        aft_sb[:, k * NP + k * 128 : k * NP + (k + 1) * 128],
                        ident[:, :],
                    )
                # NOTE: bt ((B)^T tiles) and bu (B tiles) each carry one 0.9 factor.
                ppA.__exit__(None, None, None)

                ppL = tc.tile_pool(name="psumL", bufs=1, space="PSUM")
                pp = ppL.__enter__()

                # ---- matrix squaring ladder during the AllReduce window
                # X stored as tiles [row-part, col]; X2 = lhs(X^T twin) x rhs(X)
                bt2_sb = persist.tile([128, 4 * NP], bf16, name="bt2_sb", tag="bt2_sb")
                bu2_sb = persist.tile([128, 4 * NP], bf16, name="bu2_sb", tag="bu2_sb")
                bt4_sb = persist.tile([128, 4 * NP], bf16, name="bt4_sb", tag="bt4_sb")
                bt2_t = [bt2_sb[:, k * NP : (k + 1) * NP] for k in range(4)]
                bu2_t = [bu2_sb[:, k * NP : (k + 1) * NP] for k in range(4)]
                bt4_t = [bt4_sb[:, k * NP : (k + 1) * NP] for k in range(4)]
                for dst, lhs, rhs in (
                    (bt2_t, bu_t, bt_t),
                    (bu2_t, bt_t, bu_t),
                    (bt4_t, bu2_t, bt2_t),
                ):
                    for m in range(4):
                        sq_psum = pp.tile([128, NP], f32, name="sq", tag="sq", bufs=2)
                        for k in range(4):
                            nc.tensor.matmul(
                                sq_psum[:, :],
                                lhs[k][:, m * 128 : (m + 1) * 128],
                                rhs[k][:, :],
                                start=(k == 0), stop=(k == 3),
                            )
                        if m % 2 == 0:
                            nc.scalar.copy(dst[m], sq_psum[:, :])
                        else:
                            nc.vector.tensor_copy(dst[m], sq_psum[:, :])

                ppL.__exit__(None, None, None)
                ppB = tc.tile_pool(name="psumB", bufs=1, space="PSUM")
                pp = ppB.__enter__()

                if stage <= 3:
                    resm = stream.tile([1, 16], f32, name="resm", tag="resm")
                    nc.gpsimd.memset(resm[:, :], 0.0)
                    nc.vector.tensor_copy(resm[:, 0:16], bt4_sb[0:1, 0:16])
                    nc.sync.dma_start(out_ext[:, :], resm[:, :])
                    ppB.__exit__(None, None, None)
                    break

                # ---- AllReduce readback: er as per-partition cols, el as rows
                erl = persist.tile([128, 4, 6], f32, name="erl", tag="erl")
                nc.sync.dma_start(erl[:, :, :], eler_out.rearrange("(k p) c -> p k c", p=128))
                el_rows = [
                    persist.tile([1, NP], f32, name=f"el_row{h}", tag=f"el_row{h}")
                    for h in range(H)
                ]
                for h in range(H):
                    nc.sync.dma_start(
                        el_rows[h][:, :], eler_out[:, h : h + 1].rearrange("n h -> h n")
                    )

                # ---- attention numerators, [src j, dst i] layout, bf16
                elb_psum = [pp.tile([128, NP], f32, name=f"elb{h}", tag=f"elb{h}") for h in range(H)]
                for h in range(H):
                    nc.tensor.matmul(elb_psum[h][:, :], ones_row[:, :], el_rows[h][:, :])
                num_t = {}
                for h in range(H):
                    for k in range(4):
                        z_t = stream.tile([128, NP], bf16, name="zt", tag="zt", bufs=4)
                        if h < 2:
                            nc.scalar.activation(
                                z_t[:, :], elb_psum[h][:, :], Act.Identity,
                                bias=erl[:, k, 3 + h : 4 + h],
                            )
                        else:
                            nc.vector.tensor_scalar_add(
                                z_t[:, :], elb_psum[h][:, :], erl[:, k, 3 + h : 4 + h]
                            )
                        lr_t = stream.tile([128, NP], bf16, name="lrt", tag="lrt", bufs=4)
                        nc.vector.scalar_tensor_tensor(
                            lr_t[:, :], z_t[:, :], NEG_SLOPE, z_t[:, :],
                            op0=Alu.mult, op1=Alu.max,
                        )
                        ex_t = stream.tile([128, NP], bf16, name="ext", tag="ext", bufs=4)
                        nc.scalar.activation(ex_t[:, :], lr_t[:, :], Act.Exp)
                        numb = persist.tile([128, NP], bf16, name=f"num{h}_{k}", tag=f"num{h}_{k}")
                        eng = nc.vector if k % 2 == 0 else nc.gpsimd
                        eng.tensor_mul(numb[:, :], ex_t[:, :], aft_t[k])
                        num_t[(h, k)] = numb

                if stage <= 4:
                    resm = stream.tile([1, 16], f32, name="resm", tag="resm")
                    nc.gpsimd.memset(resm[:, :], 0.0)
                    nc.vector.tensor_copy(resm[:, 0:16], num_t[(2, 3)][0:1, 0:16])
                    nc.sync.dma_start(out_ext[:, :], resm[:, :])
                    ppB.__exit__(None, None, None)
                    break

                # ---- h0 = att @ h with fused denominators (ones column per head)
                # (h0 psum lives in the same pool as elb: distinct banks, no WAR)
                h0_psum = [
                    pp.tile([128, H, 129], f32, name=f"h0p{m}", tag=f"h0p{m}")
                    for m in range(4)
                ]
                for m in range(4):
                    for h in range(H):
                        for k in range(4):
                            nc.tensor.matmul(
                                h0_psum[m][:, h, :],
                                num_t[(h, k)][:, m * 128 : (m + 1) * 128],
                                hp_sb[k][:, h, :],
                                start=(k == 0),
                                stop=(k == 3),
                            )
                h0_sb = [persist.tile([128, SH], bf16, name=f"h0sb{m}", tag=f"h0sb{m}") for m in range(4)]
                h0s = [persist.tile([128, SH], f32, name=f"h0s{m}", tag=f"h0s{m}") for m in range(4)]
                for m in range(4):
                    # denominators live in column 128 of each head block
                    rdt = stream.tile([128, H], f32, name="rdt", tag="rdt", bufs=2)
                    nc.vector.tensor_scalar_add(rdt[:, :], h0_psum[m][:, :, 128], 1e-30)
                    rd = stream.tile([128, H], f32, name="rd", tag="rd", bufs=2)
                    nc.vector.reciprocal(rd[:, :], rdt[:, :])
                    rd01 = stream.tile([128, H], f32, name="rd01", tag="rd01", bufs=2)
                    nc.vector.tensor_scalar_mul(rd01[:, :], rd[:, :], ALPHA)
                    for h in range(H):
                        nc.scalar.mul(
                            h0_sb[m][:, h * OL : (h + 1) * OL],
                            h0_psum[m][:, h, 0:128],
                            rd[:, h : h + 1],
                        )
                        nc.vector.tensor_scalar_mul(
                            h0s[m][:, h * OL : (h + 1) * OL],
                            h0_psum[m][:, h, 0:128],
                            rd01[:, h : h + 1],
                        )
                ppB.__exit__(None, None, None)

                if stage <= 5:
                    resm = stream.tile([1, 16], f32, name="resm", tag="resm")
                    nc.gpsimd.memset(resm[:, :], 0.0)
                    nc.vector.tensor_copy(resm[:, 0:16], h0_sb[0][0:1, 0:16])
                    nc.sync.dma_start(out_ext[:, :], resm[:, :])
                    
                    break

                ppD = tc.tile_pool(name="psumD", bufs=1, space="PSUM")
                pp = ppD.__enter__()

                # ---- APPNP k=10 via squaring ladder:
                # s1=B h0; g2=a*s1+a*h0; g4=g2+B^2 g2;
                # h4=B^4 h0+g4; h8=B^4 h4+g4; h10=B^2 h8+g2
                def matround(lhs_t, rhs, name, tg):
                    outp = []
                    for m in range(4):
                        r_psum = pp.tile(
                            [128, SH], f32, name=f"{name}{m}", tag=f"rnd_{tg}{m}"
                        )
                        for k in range(4):
                            nc.tensor.matmul(
                                r_psum[:, :],
                                lhs_t[k][:, m * 128 : (m + 1) * 128],
                                rhs[k][:, :],
                                start=(k == 0), stop=(k == 3),
                            )
                        outp.append(r_psum)
                    return outp

                def emit(psums, scal, addend, name, dt):
                    outs = []
                    for m in range(4):
                        t = persist.tile([128, SH], dt, name=f"{name}{m}", tag=f"{name}{m}")
                        nc.vector.scalar_tensor_tensor(
                            t[:, :], psums[m][:, :], scal, addend[m][:, :],
                            op0=Alu.mult, op1=Alu.add,
                        )
                        outs.append(t)
                    return outs

                s1p = matround(bt_t, h0_sb, "s1p", "a")
                g2 = emit(s1p, ALPHA, h0s, "g2", bf16)  # g2 = a*Bh0 + a*h0
                u4p = matround(bt4_t, h0_sb, "u4p", "b")  # B^4 h0, indep of g-chain
                t4p = matround(bt2_t, g2, "t4p", "a")
                g4 = emit(t4p, 1.0, g2, "g4", bf16)
                h4 = emit(u4p, 1.0, g4, "h4", bf16)
                h8p = matround(bt4_t, h4, "h8p", "a")
                h8 = emit(h8p, 1.0, g4, "h8", bf16)
                hXp = matround(bt2_t, h8, "hXp", "b")
                hc = emit(hXp, 1.0, g2, "hX", bf16)
                ppD.__exit__(None, None, None)

                if stage <= 6:
                    resm = stream.tile([1, 16], f32, name="resm", tag="resm")
                    nc.gpsimd.memset(resm[:, :], 0.0)
                    nc.vector.tensor_copy(resm[:, 0:16], hc[0][0:1, 0:16])
                    nc.sync.dma_start(out_ext[:, :], resm[:, :])
                    
                    break

                # ---- fc: partial dots (mul + reduce) + AllReduce + bias
                parts = stream.tile([128, 8], f32, name="parts", tag="parts")
                for m in range(4):
                    for c in range(2):
                        junk = stream.tile([128, SH], bf16, name="fcjunk", tag="fcjunk", bufs=4)
                        nc.vector.tensor_mul(
                            junk[:, :], hc[m][:, :],
                            fcw_t[m][:, c * SH : (c + 1) * SH],
                        )
                        nc.vector.reduce_sum(
                            parts[:, c * 4 + m : c * 4 + m + 1], junk[:, :], axis=AX
                        )
                ppE = tc.tile_pool(name="psumE", bufs=1, space="PSUM")
                ppe = ppE.__enter__()
                fin_psum = ppe.tile([1, 8], f32, name="fin", tag="fin")
                nc.tensor.matmul(fin_psum[:, :], ones_col[:, :], parts[:, :])
                res256 = stream.tile([1, 256], f32, name="res256", tag="res256")
                nc.gpsimd.memset(res256[:, :], 0.0)
                nc.vector.reduce_sum(
                    res256[:, 0:2], fin_psum.rearrange("p (b c) -> p b c", b=2), axis=AX
                )
                ppE.__exit__(None, None, None)
                if stage <= 7:
                    nc.sync.dma_start(out_ext[:, :], res256[0:1, 0:16])
                    break
                fc_in = dram.tile([1, 256], f32, name="fc_in", tag="fc_in")
                fc_out = dram.tile([1, 32], f32, name="fc_out", tag="fc_out")
                nc.sync.dma_start(fc_in[:, :], res256[:, :])
                # ReduceScatter: core 0 receives the reduced chunk 0, which
                # holds the two output logits; other cores' chunks are unused.
                nc.gpsimd.collective_compute(
                    "ReduceScatter", Alu.add, ins=[fc_in.opt()], outs=[fc_out.opt()],
                    replica_groups=rg,
                )
                res_f = stream.tile([1, 16], f32, name="resf", tag="resf")
                nc.sync.dma_start(res_f[:, :], fc_out[0:1, 0:16])
                nc.vector.tensor_add(res_f[:, :], res_f[:, :], fcb_sb[:, :])
                nc.sync.dma_start(out_ext[:, :], res_f[:, :])

    nc.finalize()
    return nc


def prepare_in_maps(A, x, W, attn_l, attn_r, fc_w, fc_b):
    import ml_dtypes

    bf16 = ml_dtypes.bfloat16
    A = np.asarray(A)
    x = np.asarray(x, dtype=np.float32)
    W = np.asarray(W, dtype=np.float32)
    attn_l = np.asarray(attn_l, dtype=np.float32)
    attn_r = np.asarray(attn_r, dtype=np.float32)
    fc_w = np.asarray(fc_w, dtype=np.float32)
    fc_b = np.asarray(fc_b, dtype=np.float32)

    xT = np.zeros((F, NP), dtype=bf16)
    xT[:, :N] = x.T.astype(bf16)
    # k-tile slot order [j0, j2, j1, j3] within each group (see XOFF)
    xg = np.ascontiguousarray(
        xT.reshape(NG, G, 128, NP)[:, [0, 2, 1, 3]]
        .transpose(0, 2, 1, 3)
        .reshape(NG, 128, XB)
    )
    aft = np.zeros((NP, NP), dtype=bf16)
    aft[:N, :N] = A.T.astype(bf16)
    aftp = np.ascontiguousarray(
        aft.reshape(4, 128, NP).transpose(1, 0, 2).reshape(128, 4 * NP)
    )
    af = np.zeros((NP, NP), dtype=bf16)
    af[:N, :N] = A.astype(bf16)
    afp = np.ascontiguousarray(
        af.reshape(4, 128, NP).transpose(1, 0, 2).reshape(128, 4 * NP)
    )
    fcb = np.zeros((1, 16), dtype=np.float32)
    fcb[0, :2] = fc_b
    fcv = fc_w.reshape(2, N, H, O)

    in_maps = []
    for c in range(NC):
        sl = slice(c * OL, (c + 1) * OL)
        w_c = W[:, :, sl].transpose(1, 0, 2).reshape(F, SH).astype(bf16)
        wg = np.ascontiguousarray(
            w_c.reshape(NG, G, 128, SH).transpose(0, 2, 1, 3).reshape(NG, 128, WB)
        )
        xwg = np.concatenate([xg, wg], axis=2)
        # attn broadcast tiles: [128, 768] = (l | r), (h, o) col order
        attn_c = np.concatenate(
            [attn_l[:, sl].reshape(-1), attn_r[:, sl].reshape(-1)]
        ).astype(bf16)
        attn_bc = np.ascontiguousarray(
            np.broadcast_to(attn_c.reshape(1, 2 * SH), (128, 2 * SH))
        )
        fcw_c = np.zeros((NP, 2 * SH), dtype=np.float32)
        fcw_c[:N, :] = fcv[:, :, :, sl].transpose(1, 0, 2, 3).reshape(N, 2 * SH)
        fcwp = np.ascontiguousarray(
            fcw_c.reshape(4, 128, 2 * SH).transpose(1, 0, 2).reshape(128, 8 * SH)
        ).astype(bf16)
        in_maps.append(
            {"xw": xwg, "aftp": aftp, "afp": afp, "attn": attn_bc,
             "fcwp": fcwp, "fcb": fcb}
        )
    return in_maps


def _ensure_ntff_hook():
    """The agent image's antenv lacks axon_hooks; register the profile hook
    ourselves so run_bass_kernel_spmd(trace=True) can collect NTFF profiles."""
    import types

    try:
        from antenv.axon_hooks import get_axon_ntff_profile_hook  # noqa: F401
        return
    except ImportError:
        pass
    try:
        import antenv
        from trn_agent_boot.trn_boot import _ntff_profile_via_ctypes

        mod = types.ModuleType("antenv.axon_hooks")
        _hook = [_ntff_profile_via_ctypes("/opt/axon/libaxon_pjrt.so")]
        mod.set_axon_ntff_profile_hook = lambda h: _hook.__setitem__(0, h)
        mod.get_axon_ntff_profile_hook = lambda: _hook[0]
        sys.modules["antenv.axon_hooks"] = mod
        antenv.axon_hooks = mod
    except Exception:
        pass


def kernel(A, x, W, attn_l, attn_r, fc_w, fc_b):
    global LAST_EXEC_NS, LAST_RESULT
    from concourse.bass_utils import run_bass_kernel_spmd

    if os.environ.get("BASS_TRACE"):
        _ensure_ntff_hook()

    in_maps = prepare_in_maps(A, x, W, attn_l, attn_r, fc_w, fc_b)
    nc = build()
    res = run_bass_kernel_spmd(
        nc, in_maps, core_ids=list(range(NC)),
        trace=bool(os.environ.get("BASS_TRACE")),
    )
    LAST_EXEC_NS = res.exec_time_ns
    LAST_RESULT = res
    out = res.results[0]["out"]
    return np.asarray(out).reshape(-1)[:2].astype(np.float32)

